# revision 56
# baseline (speedup 1.0000x reference)
"""Sparse attention (template/search) Trainium2 kernel.

Model (per batch b):
  qkv = x @ qkv_w.T                  -> split to q, k, v heads (12 heads, hd=64)
  template tokens   [0, 256)  attend to template keys only
  search   tokens [256, 1280) attend to all 1280 keys
  out = softmax(q k^T / 8) v   per head, concat heads, @ proj_w.T + proj_b

Sharding: data-parallel over batch, one batch per NeuronCore (8 cores).
No collectives needed.

Layout strategy per core:
  - x is transposed on-chip (PE transpose) to xT [C, NTOK] (feature-major).
  - qkv_w / proj_w transposed to wT [C, 3C], pwT [C, C].
  - q,k computed feature-major: qkT[f, tok] = qkv_w[f,:] @ xT  (q pre-scaled).
  - v computed token-major, augmented per head as [1 | 63 zeros | v]: 128 wide.
  - scores computed TRANSPOSED: S.T[tk, tq] = K_h @ Q_h.T, so softmax exp is
    elementwise on [tk partitions, tq free] and no transpose of P is needed.
  - AV: out[128, tq] = v_aug.T @ P.T accumulated over tk tiles; row 0 is the
    softmax denominator (free, from the ones column), rows 64:128 are O.T.
    This layout keeps every engine access 32-partition-aligned and puts the
    denominator at partition 0, which gpsimd.partition_broadcast requires.
  - normalize fully off the ACT queue (it is the pacing engine for exp):
    one wide DVE copy PSUM->SBUF, gpsimd partition_broadcast, DVE approx
    reciprocal, DVE multiply -> feature-major ot_all [C, NTOK] bf16.
  - proj: out[tok, c] = ot_all.T @ pwT accumulated over c_mid, + bias.

Scheduling: attention is ACT(exp)-paced, so PE work is software-pipelined
into it — each pair's search loop drains a "pending" list (next pair's q/k
chunks; v token tiles 2..9 during pair 0; deferred proj_w transposes during
pair 5). All matmuls run bf16 (1 cycle/row on PE vs 4 for fp32), fp32 PSUM.
"""

import numpy as np

import concourse.bacc as bacc
import concourse.mybir as mybir
import concourse.tile as tile
from concourse.masks import make_identity

P = 128
NTOK = 1280
C = 768
H = 12
HD = 64
NT = 256          # template tokens  [0, NT)
TT = NTOK // P    # 10 token tiles
CT = C // P       # 6 channel tiles
FT = 3 * C // P   # 18 qkv output tiles
SCALE = HD ** -0.5

F32 = mybir.dt.float32
BF16 = mybir.dt.bfloat16
EXP = mybir.ActivationFunctionType.Exp
MULT = mybir.AluOpType.mult
ADD = mybir.AluOpType.add


def build_nc():
    from contextlib import ExitStack

    nc = bacc.Bacc("TRN2", target_bir_lowering=False, debug=False, num_devices=8)
    x_ext = nc.dram_tensor("x", [NTOK, C], F32, kind="ExternalInput")
    w_ext = nc.dram_tensor("qkv_w", [3 * C, C], F32, kind="ExternalInput")
    pw_ext = nc.dram_tensor("proj_w", [C, C], F32, kind="ExternalInput")
    pb_ext = nc.dram_tensor("proj_b", [1, C], F32, kind="ExternalInput")
    out_ext = nc.dram_tensor("out", [NTOK, C], F32, kind="ExternalOutput")

    with tile.TileContext(nc) as tc, ExitStack() as ctx:
        const = ctx.enter_context(tc.tile_pool(name="const", bufs=1))
        ps_mm = ctx.enter_context(tc.tile_pool(name="ps_mm", bufs=4, space="PSUM"))
        ps_ot = ctx.enter_context(tc.tile_pool(name="ps_ot", bufs=2, space="PSUM"))
        big = ctx.enter_context(tc.tile_pool(name="big", bufs=1))

        ident = const.tile([P, P], F32)
        make_identity(nc, ident)
        # HAM warmup: keep the PE busy during the initial input-DMA wait so
        # its clock gate opens (1.2 -> 2.4 GHz) before the real transpose and
        # qkv stream begins.  ident.T == ident, and writing it back makes the
        # chain live (not DCE-able) and orders warmup before first real use.
        warm_ps = ps_mm.tile([P, 512], F32, tag="mm")
        for i in range(48):
            nc.tensor.transpose(warm_ps[:, :P], ident[:], ident[:])
        nc.vector.tensor_copy(ident[:], warm_ps[:, :P])
        bias_bc = const.tile([P, C], F32)
        bias_row = const.tile([1, C], F32)
        nc.sync.dma_start(bias_row[:], pb_ext.ap())
        nc.gpsimd.partition_broadcast(bias_bc[:], bias_row[0:1, :])

        xT = big.tile([P, CT, NTOK], BF16)     # x.T  (feature-major x)
        wT = big.tile([P, CT, 3 * C], BF16)    # qkv_w.T
        pwT = big.tile([P, CT, C], BF16)       # proj_w.T

        def transpose_blocks(srcs, dst_full):
            """srcs: list of [128,128] f32 SBUF APs; dst_full: [128, len*128]
            bf16 AP, contiguous. PE-transpose each block, copy out in groups
            of up to 4 (amortizes the PSUM->SBUF copy)."""
            i = 0
            while i < len(srcs):
                n = min(4, len(srcs) - i)
                pt = ps_mm.tile([P, 512], F32, tag="mm")
                for j in range(n):
                    nc.tensor.transpose(
                        pt[:, j * P:(j + 1) * P], srcs[i + j], ident[:]
                    )
                nc.vector.tensor_copy(
                    dst_full[:, i * P:(i + n) * P], pt[:, : n * P]
                )
                i += n

        with tc.tile_pool(name="staging", bufs=2) as staging:
            # x group 0, then the two w groups holding q/k weights, then the
            # second x group, then v weights: gets pair-0 q/k built earliest
            def emit_xg(g):
                xg = staging.tile([P, 5, C], F32, tag="xg", name=f"xg{g}")
                for j in range(5):
                    t0 = (g * 5 + j) * P
                    nc.sync.dma_start(xg[:, j, :], x_ext.ap()[t0:t0 + P, :])
                for ct in range(CT):
                    transpose_blocks(
                        [xg[:, j, ct * P:(ct + 1) * P] for j in range(5)],
                        xT[:, ct, g * 5 * P:(g * 5 + 5) * P],
                    )

            def emit_wg(g):
                wg = staging.tile([P, 6, C], F32, tag="wg", name=f"wg{g}")
                for j in range(6):
                    f0 = (g * 6 + j) * P
                    nc.sync.dma_start(wg[:, j, :], w_ext.ap()[f0:f0 + P, :])
                for ct in range(CT):
                    transpose_blocks(
                        [wg[:, j, ct * P:(ct + 1) * P] for j in range(6)],
                        wT[:, ct, g * 6 * P:(g * 6 + 6) * P],
                    )

            emit_xg(0)
            emit_wg(0)
            emit_wg(1)
            emit_xg(1)
            emit_wg(2)
            # ---- proj_w: DMA now, transpose later (filler work for the
            # last attention pair, which has no next-pair qk chunks) ----
            pg = big.tile([P, 6, C], F32)
            for j in range(6):
                nc.sync.dma_start(pg[:, j, :], pw_ext.ap()[j * P:(j + 1) * P, :])

        big2 = ctx.enter_context(tc.tile_pool(name="big2", bufs=1))
        qk = big2.tile([P, 2 * CT, NTOK], BF16)     # [q (scaled) | k] feature-major
        v_sb = big2.tile([P, TT, H, P], BF16)  # [1 | 63 zeros | v]: denom row 0, O rows 64:128
        ot_all = big2.tile([P, CT, NTOK], BF16)     # attention out, feature-major
        out_sb = big2.tile([P, TT, C], F32)

        # v_aug layout per head: col 0 = ones (softmax denominator row),
        # cols 1:32 = zeros (padding so O lands 32-aligned), cols 32:96 = v
        nc.gpsimd.memset(v_sb[:, :, :, 0:64], 0.0)
        nc.gpsimd.memset(v_sb[:, :, :, 0:1], 1.0)

        # ---- qkv projection (emitted interleaved with attention below) ----
        def emit_qk_chunk(ft, c0, cw):
            """qk[f, tok] = qkv_w @ x.T rows [0, 1536) for one (ftile, chunk)."""
            ps = ps_mm.tile([P, 512], F32, tag="mm", name=f"qkp{ft}_{c0}")
            for ct in range(CT):
                nc.tensor.matmul(
                    ps[:, :cw],
                    wT[:, ct, ft * P:(ft + 1) * P],
                    xT[:, ct, c0:c0 + cw],
                    start=(ct == 0), stop=(ct == CT - 1),
                )
            if ft < 6:  # q: fold in softmax scale
                nc.vector.tensor_scalar_mul(
                    qk[:, ft, c0:c0 + cw], ps[:, :cw], SCALE
                )
            else:
                nc.vector.tensor_copy(qk[:, ft, c0:c0 + cw], ps[:, :cw])

        def qk_pair_chunks(p):
            # ft-interleaved so the (q, k) chunks a consumer needs first come
            # out adjacent: template needs both c0 chunks only
            return [(ft, c0, cw)
                    for c0, cw in ((0, 512), (512, 512), (1024, 256))
                    for ft in (p, 6 + p)]

        def emit_filler(kind, arg):
            if kind == "qk":
                emit_qk_chunk(*arg)
            elif kind == "v":
                emit_v_chunk(*arg)
            else:  # deferred proj_w transpose for channel tile `arg`
                transpose_blocks(
                    [pg[:, j, arg * P:(arg + 1) * P] for j in range(6)],
                    pwT[:, arg, :],
                )

        # q/k for head pair 0 up front
        for ft, c0, cw in qk_pair_chunks(0):
            emit_qk_chunk(ft, c0, cw)

        # v token-major: v[tok, f] = x @ qkv_w.T cols [1536, 2304)
        def emit_v_chunk(tt, half):
            c0, cw, h0, nh = ((0, 512, 0, 8), (512, 256, 8, 4))[half]
            ps = ps_mm.tile([P, 512], F32, tag="mm", name=f"vp{tt}_{half}")
            for ct in range(CT):
                nc.tensor.matmul(
                    ps[:, :cw],
                    xT[:, ct, tt * P:(tt + 1) * P],
                    wT[:, ct, 2 * C + c0:2 * C + c0 + cw],
                    start=(ct == 0), stop=(ct == CT - 1),
                )
            nc.vector.tensor_copy(
                v_sb[:, tt, h0:h0 + nh, 64:128],
                ps[:, :cw].rearrange("p (h e) -> p h e", e=HD),
            )

        # only the first two token tiles of v are needed before pair 0 starts
        # (template reads them); the rest stream in as pair-0 filler work
        for tt in (0, 1):
            emit_v_chunk(tt, 0)
            emit_v_chunk(tt, 1)

        # ---- attention ----
        pts = ctx.enter_context(tc.tile_pool(name="pts", bufs=8))
        dn = ctx.enter_context(tc.tile_pool(name="dn", bufs=2))
        rbp = ctx.enter_context(tc.tile_pool(name="rbp", bufs=2))

        def qh(h, c0, cw):
            b = (h % 2) * 64
            return qk[b:b + 64, h // 2, c0:c0 + cw]

        def kh(h, tk):
            b = (h % 2) * 64
            return qk[b:b + 64, 6 + h // 2, tk * P:(tk + 1) * P]

        def normalize(h, ot_ps, c0, cw):
            """ot_ps: [128, cw] psum (row 0 = denominators, rows 64:128 = O.T
            for tq cols [c0, c0+cw)). Normalize and write to ot_all. The whole
            tile is lifted PSUM->SBUF by one wide DVE copy, so the in-order
            ACT queue (busy with exps) is never involved and the PSUM slot
            frees early."""
            b = (h % 2) * 64
            den = dn.tile([P, 1024], F32, tag="dn")
            nc.vector.tensor_copy(den[:, :cw], ot_ps[:, :cw])
            rb = rbp.tile([P, 1024], F32, tag="rb")
            nc.gpsimd.partition_broadcast(rb[:, :cw], den[0:1, :cw])
            # approx reciprocal (~18 bits, plenty for bf16 outputs) on 128 lanes
            nc.vector.reciprocal_approx_fast(rb[:, :cw], rb[:, :cw])
            nc.vector.tensor_tensor(
                ot_all[b:b + 64, h // 2, c0:c0 + cw],
                den[64:128, :cw], rb[64:128, :cw], MULT,
            )

        def emit_template(h):
            """Template block: queries [0,256) attend keys [0,256)."""
            st = ps_mm.tile([P, 512], F32, tag="mm", name=f"tst{h}")
            for tj in range(2):
                nc.tensor.matmul(
                    st[:, tj * NT:(tj + 1) * NT], kh(h, tj),
                    qh(h, 0, NT), start=True, stop=True,
                )
            pt = pts.tile([P, 512], BF16, tag="pt", name=f"tpt{h}")
            nc.scalar.activation(pt[:], st[:], EXP)
            to = ps_ot.tile([P, 1024], F32, tag="ot", name=f"tot{h}")
            for tj in range(2):
                nc.tensor.matmul(
                    to[:, :NT], v_sb[:, tj, h, :],
                    pt[:, tj * NT:(tj + 1) * NT],
                    start=(tj == 0), stop=(tj == 1),
                )
            normalize(h, to, 0, NT)

        for hp in range(6):
            pair = (2 * hp, 2 * hp + 1)
            # qk chunks of the NEXT pair, fed into PE idle slots while this
            # pair's ACT-bound attention runs.  The last pair gets the
            # deferred proj_w transposes as its filler instead.
            pending = [("qk", a) for a in qk_pair_chunks(hp + 1)] if hp < 5 \
                else [("pw", ct) for ct in range(CT)]
            if hp == 0:
                # v tiles 2..9 stream just-in-time ahead of their AV use
                pending = [("v", (tt, half)) for tt in range(2, TT)
                           for half in (0, 1)] + pending
            # template block for both heads (queries [0,256) x keys [0,256))
            for h in pair:
                emit_template(h)
            # search: queries [256, 1280) attend all keys
            ots = {h: ps_ot.tile([P, 1024], F32, tag="ot", name=f"ot_s{h}") for h in pair}
            for tk in range(TT):
                sts, pts_ = {}, {}
                # S^T matmuls for both heads back-to-back: the two heads sit
                # on PE row groups 0-63 / 64-127, so the array packs them.
                for h in pair:
                    for cj in range(2):
                        st = sts[h, cj] = ps_mm.tile(
                            [P, 512], F32, tag="mm", name=f"st{h}_{cj}")
                        nc.tensor.matmul(
                            st[:], kh(h, tk), qh(h, NT + cj * 512, 512),
                            start=True, stop=True)
                for h in pair:
                    for cj in range(2):
                        pt = pts_[h, cj] = pts.tile(
                            [P, 512], BF16, tag="pt", name=f"pt{h}_{cj}")
                        nc.scalar.activation(pt[:], sts[h, cj][:], EXP)
                for h in pair:
                    for cj in range(2):
                        nc.tensor.matmul(
                            ots[h][:, cj * 512:(cj + 1) * 512],
                            v_sb[:, tk, h, :], pts_[h, cj][:],
                            start=(tk == 0), stop=(tk == TT - 1),
                        )
                # feed filler into the PE stream (pair 0 carries the v tail
                # and needs a higher drain rate to stay ahead of its AVs;
                # later pairs spread their 6 items across the whole loop)
                if hp == 0:
                    for _ in range(3):
                        if pending:
                            emit_filler(*pending.pop(0))
                elif tk % 2 == 0 and pending:
                    emit_filler(*pending.pop(0))
            for h in pair:
                normalize(h, ots[h], NT, NTOK - NT)
            while pending:
                emit_filler(*pending.pop(0))

        # ---- output projection ----
        def emit_proj(tt):
            for c0, cw in ((0, 512), (512, 256)):
                ps = ps_mm.tile([P, 512], F32, tag="mm", name=f"prj{tt}_{c0}")
                for ct in range(CT):
                    nc.tensor.matmul(
                        ps[:, :cw],
                        ot_all[:, ct, tt * P:(tt + 1) * P],
                        pwT[:, ct, c0:c0 + cw],
                        start=(ct == 0), stop=(ct == CT - 1),
                    )
                nc.vector.tensor_tensor(
                    out_sb[:, tt, c0:c0 + cw], ps[:, :cw],
                    bias_bc[:, c0:c0 + cw], ADD,
                )
            nc.sync.dma_start(out_ext.ap()[tt * P:(tt + 1) * P, :],
                              out_sb[:, tt, :])

        for tt in range(TT):
            emit_proj(tt)

    nc.compile()
    return nc


_NC = None


def _get_nc():
    global _NC
    if _NC is None:
        _NC = build_nc()
    return _NC


def kernel(x, qkv_w, proj_w, proj_b, **_ignored):
    from concourse.bass_utils import run_bass_kernel_spmd

    x = np.ascontiguousarray(np.asarray(x), dtype=np.float32)
    qkv_w = np.ascontiguousarray(np.asarray(qkv_w), dtype=np.float32)
    proj_w = np.ascontiguousarray(np.asarray(proj_w), dtype=np.float32)
    proj_b = np.ascontiguousarray(np.asarray(proj_b), dtype=np.float32).reshape(1, C)

    nc = _get_nc()
    in_maps = [
        {"x": x[i], "qkv_w": qkv_w, "proj_w": proj_w, "proj_b": proj_b}
        for i in range(8)
    ]
    res = run_bass_kernel_spmd(nc, in_maps, list(range(8)))
    return np.stack([res.results[i]["out"] for i in range(8)])


if __name__ == "__main__":
    rng = np.random.default_rng(0)
    ins = {
        "x": rng.standard_normal((8, NTOK, C), dtype=np.float32),
        "qkv_w": rng.standard_normal((3 * C, C), dtype=np.float32) * 0.02,
        "proj_w": rng.standard_normal((C, C), dtype=np.float32) * 0.02,
        "proj_b": np.zeros(C, dtype=np.float32),
    }
    out = kernel(**ins)
    print("out", out.shape, out.dtype)


# revision 57
# speedup vs baseline: 1.0274x; 1.0274x over previous
"""Sparse attention (template/search) Trainium2 kernel.

Model (per batch b):
  qkv = x @ qkv_w.T                  -> split to q, k, v heads (12 heads, hd=64)
  template tokens   [0, 256)  attend to template keys only
  search   tokens [256, 1280) attend to all 1280 keys
  out = softmax(q k^T / 8) v   per head, concat heads, @ proj_w.T + proj_b

Sharding: data-parallel over batch, one batch per NeuronCore (8 cores).
No collectives needed.

Layout strategy per core:
  - x is transposed on-chip (PE transpose) to xT [C, NTOK] (feature-major).
  - qkv_w / proj_w transposed to wT [C, 3C], pwT [C, C].
  - q,k computed feature-major: qkT[f, tok] = qkv_w[f,:] @ xT  (q pre-scaled).
  - v computed token-major, augmented per head as [1 | 63 zeros | v]: 128 wide.
  - scores computed TRANSPOSED: S.T[tk, tq] = K_h @ Q_h.T, so softmax exp is
    elementwise on [tk partitions, tq free] and no transpose of P is needed.
  - AV: out[128, tq] = v_aug.T @ P.T accumulated over tk tiles; row 0 is the
    softmax denominator (free, from the ones column), rows 64:128 are O.T.
    This layout keeps every engine access 32-partition-aligned and puts the
    denominator at partition 0, which gpsimd.partition_broadcast requires.
  - normalize fully off the ACT queue (it is the pacing engine for exp):
    one wide DVE copy PSUM->SBUF, gpsimd partition_broadcast, DVE approx
    reciprocal, DVE multiply -> feature-major ot_all [C, NTOK] bf16.
  - proj: out[tok, c] = ot_all.T @ pwT accumulated over c_mid, + bias.

Scheduling: attention is ACT(exp)-paced, so PE work is software-pipelined
into it — each pair's search loop drains a "pending" list (next pair's q/k
chunks; v token tiles 2..9 during pair 0; deferred proj_w transposes during
pair 5). All matmuls run bf16 (1 cycle/row on PE vs 4 for fp32), fp32 PSUM.
"""

import numpy as np

import concourse.bacc as bacc
import concourse.mybir as mybir
import concourse.tile as tile
from concourse.masks import make_identity

P = 128
NTOK = 1280
C = 768
H = 12
HD = 64
NT = 256          # template tokens  [0, NT)
TT = NTOK // P    # 10 token tiles
CT = C // P       # 6 channel tiles
FT = 3 * C // P   # 18 qkv output tiles
SCALE = HD ** -0.5

F32 = mybir.dt.float32
BF16 = mybir.dt.bfloat16
EXP = mybir.ActivationFunctionType.Exp
MULT = mybir.AluOpType.mult
ADD = mybir.AluOpType.add


def build_nc():
    from contextlib import ExitStack

    nc = bacc.Bacc("TRN2", target_bir_lowering=False, debug=False, num_devices=8)
    x_ext = nc.dram_tensor("x", [NTOK, C], F32, kind="ExternalInput")
    w_ext = nc.dram_tensor("qkv_w", [3 * C, C], F32, kind="ExternalInput")
    pw_ext = nc.dram_tensor("proj_w", [C, C], F32, kind="ExternalInput")
    pb_ext = nc.dram_tensor("proj_b", [1, C], F32, kind="ExternalInput")
    out_ext = nc.dram_tensor("out", [NTOK, C], F32, kind="ExternalOutput")

    with tile.TileContext(nc) as tc, ExitStack() as ctx:
        const = ctx.enter_context(tc.tile_pool(name="const", bufs=1))
        ps_mm = ctx.enter_context(tc.tile_pool(name="ps_mm", bufs=4, space="PSUM"))
        ps_ot = ctx.enter_context(tc.tile_pool(name="ps_ot", bufs=2, space="PSUM"))
        big = ctx.enter_context(tc.tile_pool(name="big", bufs=1))

        ident = const.tile([P, P], F32)
        make_identity(nc, ident)
        # HAM warmup: keep the PE busy during the initial input-DMA wait so
        # its clock gate opens (1.2 -> 2.4 GHz) before the real transpose and
        # qkv stream begins.  ident.T == ident, and writing it back makes the
        # chain live (not DCE-able) and orders warmup before first real use.
        warm_ps = ps_mm.tile([P, 512], F32, tag="mm")
        for i in range(24):
            nc.tensor.transpose(warm_ps[:, :P], ident[:], ident[:])
        nc.vector.tensor_copy(ident[:], warm_ps[:, :P])
        bias_bc = const.tile([P, C], F32)
        bias_row = const.tile([1, C], F32)
        nc.sync.dma_start(bias_row[:], pb_ext.ap())
        nc.gpsimd.partition_broadcast(bias_bc[:], bias_row[0:1, :])

        xT = big.tile([P, CT, NTOK], BF16)     # x.T  (feature-major x)
        wT = big.tile([P, CT, 3 * C], BF16)    # qkv_w.T
        pwT = big.tile([P, CT, C], BF16)       # proj_w.T

        def transpose_blocks(srcs, dst_full):
            """srcs: list of [128,128] f32 SBUF APs; dst_full: [128, len*128]
            bf16 AP, contiguous. PE-transpose each block, copy out in groups
            of up to 4 (amortizes the PSUM->SBUF copy)."""
            i = 0
            while i < len(srcs):
                n = min(4, len(srcs) - i)
                pt = ps_mm.tile([P, 512], F32, tag="mm")
                for j in range(n):
                    nc.tensor.transpose(
                        pt[:, j * P:(j + 1) * P], srcs[i + j], ident[:]
                    )
                nc.vector.tensor_copy(
                    dst_full[:, i * P:(i + n) * P], pt[:, : n * P]
                )
                i += n

        with tc.tile_pool(name="staging", bufs=2) as staging:
            # x group 0, then the two w groups holding q/k weights, then the
            # second x group, then v weights: gets pair-0 q/k built earliest
            def emit_xg(g):
                xg = staging.tile([P, 5, C], F32, tag="xg", name=f"xg{g}")
                for j in range(5):
                    t0 = (g * 5 + j) * P
                    nc.sync.dma_start(xg[:, j, :], x_ext.ap()[t0:t0 + P, :])
                for ct in range(CT):
                    transpose_blocks(
                        [xg[:, j, ct * P:(ct + 1) * P] for j in range(5)],
                        xT[:, ct, g * 5 * P:(g * 5 + 5) * P],
                    )

            def emit_wg(g):
                wg = staging.tile([P, 6, C], F32, tag="wg", name=f"wg{g}")
                for j in range(6):
                    f0 = (g * 6 + j) * P
                    nc.sync.dma_start(wg[:, j, :], w_ext.ap()[f0:f0 + P, :])
                for ct in range(CT):
                    transpose_blocks(
                        [wg[:, j, ct * P:(ct + 1) * P] for j in range(6)],
                        wT[:, ct, g * 6 * P:(g * 6 + 6) * P],
                    )

            emit_xg(0)
            emit_wg(0)
            emit_wg(1)
            emit_xg(1)
            emit_wg(2)
            # ---- proj_w: DMA now, transpose later (filler work for the
            # last attention pair, which has no next-pair qk chunks) ----
            pg = big.tile([P, 6, C], F32)
            for j in range(6):
                nc.sync.dma_start(pg[:, j, :], pw_ext.ap()[j * P:(j + 1) * P, :])

        big2 = ctx.enter_context(tc.tile_pool(name="big2", bufs=1))
        qk = big2.tile([P, 2 * CT, NTOK], BF16)     # [q (scaled) | k] feature-major
        v_sb = big2.tile([P, TT, H, P], BF16)  # [1 | 63 zeros | v]: denom row 0, O rows 64:128
        ot_all = big2.tile([P, CT, NTOK], BF16)     # attention out, feature-major
        out_sb = big2.tile([P, TT, C], F32)

        # v_aug layout per head: col 0 = ones (softmax denominator row),
        # cols 1:32 = zeros (padding so O lands 32-aligned), cols 32:96 = v
        nc.gpsimd.memset(v_sb[:, :, :, 0:64], 0.0)
        nc.gpsimd.memset(v_sb[:, :, :, 0:1], 1.0)

        # ---- qkv projection (emitted interleaved with attention below) ----
        def emit_qk_chunk(ft, c0, cw):
            """qk[f, tok] = qkv_w @ x.T rows [0, 1536) for one (ftile, chunk)."""
            ps = ps_mm.tile([P, 512], F32, tag="mm", name=f"qkp{ft}_{c0}")
            for ct in range(CT):
                nc.tensor.matmul(
                    ps[:, :cw],
                    wT[:, ct, ft * P:(ft + 1) * P],
                    xT[:, ct, c0:c0 + cw],
                    start=(ct == 0), stop=(ct == CT - 1),
                )
            if ft < 6:  # q: fold in softmax scale
                nc.vector.tensor_scalar_mul(
                    qk[:, ft, c0:c0 + cw], ps[:, :cw], SCALE
                )
            else:
                nc.vector.tensor_copy(qk[:, ft, c0:c0 + cw], ps[:, :cw])

        def qk_pair_chunks(p):
            # ft-interleaved so the (q, k) chunks a consumer needs first come
            # out adjacent: template needs both c0 chunks only
            return [(ft, c0, cw)
                    for c0, cw in ((0, 512), (512, 512), (1024, 256))
                    for ft in (p, 6 + p)]

        def emit_filler(kind, arg):
            if kind == "qk":
                emit_qk_chunk(*arg)
            elif kind == "v":
                emit_v_chunk(*arg)
            else:  # deferred proj_w transpose for channel tile `arg`
                transpose_blocks(
                    [pg[:, j, arg * P:(arg + 1) * P] for j in range(6)],
                    pwT[:, arg, :],
                )

        # q/k for head pair 0 up front
        for ft, c0, cw in qk_pair_chunks(0):
            emit_qk_chunk(ft, c0, cw)

        # v token-major: v[tok, f] = x @ qkv_w.T cols [1536, 2304)
        def emit_v_chunk(tt, half):
            c0, cw, h0, nh = ((0, 512, 0, 8), (512, 256, 8, 4))[half]
            ps = ps_mm.tile([P, 512], F32, tag="mm", name=f"vp{tt}_{half}")
            for ct in range(CT):
                nc.tensor.matmul(
                    ps[:, :cw],
                    xT[:, ct, tt * P:(tt + 1) * P],
                    wT[:, ct, 2 * C + c0:2 * C + c0 + cw],
                    start=(ct == 0), stop=(ct == CT - 1),
                )
            nc.vector.tensor_copy(
                v_sb[:, tt, h0:h0 + nh, 64:128],
                ps[:, :cw].rearrange("p (h e) -> p h e", e=HD),
            )

        # only the first two token tiles of v are needed before pair 0 starts
        # (template reads them); the rest stream in as pair-0 filler work
        for tt in (0, 1):
            emit_v_chunk(tt, 0)
            emit_v_chunk(tt, 1)

        # ---- attention ----
        pts = ctx.enter_context(tc.tile_pool(name="pts", bufs=8))
        dn = ctx.enter_context(tc.tile_pool(name="dn", bufs=2))
        rbp = ctx.enter_context(tc.tile_pool(name="rbp", bufs=2))

        def qh(h, c0, cw):
            b = (h % 2) * 64
            return qk[b:b + 64, h // 2, c0:c0 + cw]

        def kh(h, tk):
            b = (h % 2) * 64
            return qk[b:b + 64, 6 + h // 2, tk * P:(tk + 1) * P]

        def normalize(h, ot_ps, c0, cw):
            """ot_ps: [128, cw] psum (row 0 = denominators, rows 64:128 = O.T
            for tq cols [c0, c0+cw)). Normalize and write to ot_all. The whole
            tile is lifted PSUM->SBUF by one wide DVE copy, so the in-order
            ACT queue (busy with exps) is never involved and the PSUM slot
            frees early."""
            b = (h % 2) * 64
            den = dn.tile([P, 1024], F32, tag="dn")
            nc.vector.tensor_copy(den[:, :cw], ot_ps[:, :cw])
            rb = rbp.tile([P, 1024], F32, tag="rb")
            nc.gpsimd.partition_broadcast(rb[:, :cw], den[0:1, :cw])
            # approx reciprocal (~18 bits, plenty for bf16 outputs) on 128 lanes
            nc.vector.reciprocal_approx_fast(rb[:, :cw], rb[:, :cw])
            nc.vector.tensor_tensor(
                ot_all[b:b + 64, h // 2, c0:c0 + cw],
                den[64:128, :cw], rb[64:128, :cw], MULT,
            )

        def emit_template(h):
            """Template block: queries [0,256) attend keys [0,256)."""
            st = ps_mm.tile([P, 512], F32, tag="mm", name=f"tst{h}")
            for tj in range(2):
                nc.tensor.matmul(
                    st[:, tj * NT:(tj + 1) * NT], kh(h, tj),
                    qh(h, 0, NT), start=True, stop=True,
                )
            pt = pts.tile([P, 512], BF16, tag="pt", name=f"tpt{h}")
            nc.scalar.activation(pt[:], st[:], EXP)
            to = ps_ot.tile([P, 1024], F32, tag="ot", name=f"tot{h}")
            for tj in range(2):
                nc.tensor.matmul(
                    to[:, :NT], v_sb[:, tj, h, :],
                    pt[:, tj * NT:(tj + 1) * NT],
                    start=(tj == 0), stop=(tj == 1),
                )
            normalize(h, to, 0, NT)

        for hp in range(6):
            pair = (2 * hp, 2 * hp + 1)
            # qk chunks of the NEXT pair, fed into PE idle slots while this
            # pair's ACT-bound attention runs.  The last pair gets the
            # deferred proj_w transposes as its filler instead.
            pending = [("qk", a) for a in qk_pair_chunks(hp + 1)] if hp < 5 \
                else [("pw", ct) for ct in range(CT)]
            if hp == 0:
                # v tiles 2..9 stream just-in-time ahead of their AV use
                pending = [("v", (tt, half)) for tt in range(2, TT)
                           for half in (0, 1)] + pending
            # template block for both heads (queries [0,256) x keys [0,256))
            for h in pair:
                emit_template(h)
            # search: queries [256, 1280) attend all keys
            ots = {h: ps_ot.tile([P, 1024], F32, tag="ot", name=f"ot_s{h}") for h in pair}
            for tk in range(TT):
                sts, pts_ = {}, {}
                # S^T matmuls for both heads back-to-back: the two heads sit
                # on PE row groups 0-63 / 64-127, so the array packs them.
                for h in pair:
                    for cj in range(2):
                        st = sts[h, cj] = ps_mm.tile(
                            [P, 512], F32, tag="mm", name=f"st{h}_{cj}")
                        nc.tensor.matmul(
                            st[:], kh(h, tk), qh(h, NT + cj * 512, 512),
                            start=True, stop=True)
                for h in pair:
                    for cj in range(2):
                        pt = pts_[h, cj] = pts.tile(
                            [P, 512], BF16, tag="pt", name=f"pt{h}_{cj}")
                        nc.scalar.activation(pt[:], sts[h, cj][:], EXP)
                for h in pair:
                    for cj in range(2):
                        nc.tensor.matmul(
                            ots[h][:, cj * 512:(cj + 1) * 512],
                            v_sb[:, tk, h, :], pts_[h, cj][:],
                            start=(tk == 0), stop=(tk == TT - 1),
                        )
                # feed filler into the PE stream (pair 0 carries the v tail
                # and needs a higher drain rate to stay ahead of its AVs)
                for _ in range(3 if hp == 0 else 1):
                    if pending:
                        emit_filler(*pending.pop(0))
            for h in pair:
                normalize(h, ots[h], NT, NTOK - NT)
            while pending:
                emit_filler(*pending.pop(0))

        # ---- output projection ----
        def emit_proj(tt):
            for c0, cw in ((0, 512), (512, 256)):
                ps = ps_mm.tile([P, 512], F32, tag="mm", name=f"prj{tt}_{c0}")
                for ct in range(CT):
                    nc.tensor.matmul(
                        ps[:, :cw],
                        ot_all[:, ct, tt * P:(tt + 1) * P],
                        pwT[:, ct, c0:c0 + cw],
                        start=(ct == 0), stop=(ct == CT - 1),
                    )
                nc.vector.tensor_tensor(
                    out_sb[:, tt, c0:c0 + cw], ps[:, :cw],
                    bias_bc[:, c0:c0 + cw], ADD,
                )
            nc.sync.dma_start(out_ext.ap()[tt * P:(tt + 1) * P, :],
                              out_sb[:, tt, :])

        for tt in range(TT):
            emit_proj(tt)

    nc.compile()
    return nc


_NC = None


def _get_nc():
    global _NC
    if _NC is None:
        _NC = build_nc()
    return _NC


def kernel(x, qkv_w, proj_w, proj_b, **_ignored):
    from concourse.bass_utils import run_bass_kernel_spmd

    x = np.ascontiguousarray(np.asarray(x), dtype=np.float32)
    qkv_w = np.ascontiguousarray(np.asarray(qkv_w), dtype=np.float32)
    proj_w = np.ascontiguousarray(np.asarray(proj_w), dtype=np.float32)
    proj_b = np.ascontiguousarray(np.asarray(proj_b), dtype=np.float32).reshape(1, C)

    nc = _get_nc()
    in_maps = [
        {"x": x[i], "qkv_w": qkv_w, "proj_w": proj_w, "proj_b": proj_b}
        for i in range(8)
    ]
    res = run_bass_kernel_spmd(nc, in_maps, list(range(8)))
    return np.stack([res.results[i]["out"] for i in range(8)])


if __name__ == "__main__":
    rng = np.random.default_rng(0)
    ins = {
        "x": rng.standard_normal((8, NTOK, C), dtype=np.float32),
        "qkv_w": rng.standard_normal((3 * C, C), dtype=np.float32) * 0.02,
        "proj_w": rng.standard_normal((C, C), dtype=np.float32) * 0.02,
        "proj_b": np.zeros(C, dtype=np.float32),
    }
    out = kernel(**ins)
    print("out", out.shape, out.dtype)
